# revision 3
# baseline (speedup 1.0000x reference)
"""Trainium2 Bass kernel for Bahdanau-style attention.

reference:
    x[s,b,u]  = enc[s,b,:] @ Ua_w[u,:] + Ua_b[u] + dec[b,:] @ Wa_w[u,:] + Wa_b[u]
    att[b,s]  = softmax_s( sum_u v[u] * tanh(x[s,b,u]) )

Sharding: data-parallel over batch. 8 cores x 8 batches each; weights
replicated. All shapes hardcoded (S=512, B=64, H=2048, U=1024).

Per-core device program (bf16 matmul operands, fp32 accumulation):
  1. D[u, b]  = Wa_w.T-augmented @ dec.T-augmented   (bias folded via an
     extra contraction row), kept as ACT bias table [128, 64] in SBUF.
  2. for each local batch b (rows are b-major so the dec-projection is
     constant along the moving free dim):
       for each u-tile (8 of 128):
         psum[128u, 512s] = sum_hc UaT[hc,u-tile].T @ enc[hc, b-rows]
         energy = tanh(psum + D[:, u-tile, b])      (ScalarE bias port)
       att[1, 512] = sum_ut v[u-tile].T @ energy    (PE, M=1)
  3. softmax over s on [8, 512], DMA out.
"""

import os
import numpy as np
import ml_dtypes

BF16 = ml_dtypes.bfloat16

S = 512          # src len
B = 64           # global batch
H = 2048         # encoder hidden (2*HIDDEN)
HD = 1024        # decoder hidden
U = 1024         # attention units
NCORES = 8
BL = B // NCORES  # local batch per core = 8

HC = H // 128     # 16 h-chunks for main contraction
UT = U // 128     # 8 u-tiles
HA = HD + 128     # augmented dec contraction (1024 + bias row + pad) = 1152
HAC = HA // 128   # 9 chunks

_BUILT = None     # (nc,) cache so repeated kernel() calls reuse the program


def _build_bass():
    import concourse.mybir as mybir
    from concourse import bacc
    from concourse.tile import TileContext

    f32 = mybir.dt.float32
    bf16 = mybir.dt.bfloat16
    Tanh = mybir.ActivationFunctionType.Tanh
    Exp = mybir.ActivationFunctionType.Exp
    X = mybir.AxisListType.X

    nc = bacc.Bacc("TRN2", num_devices=NCORES)

    enc_d = nc.dram_tensor("enc", [H, BL * S], bf16, kind="ExternalInput")
    uat_d = nc.dram_tensor("uat", [H, U], bf16, kind="ExternalInput")
    wat_d = nc.dram_tensor("wat", [HA, U], bf16, kind="ExternalInput")
    dect_d = nc.dram_tensor("dect", [HA, BL], bf16, kind="ExternalInput")
    v_d = nc.dram_tensor("v", [128, UT], bf16, kind="ExternalInput")
    out_d = nc.dram_tensor("out", [BL, S], f32, kind="ExternalOutput")

    with TileContext(nc) as tc:
        with (
            tc.tile_pool(name="const", bufs=1) as const,
            tc.tile_pool(name="encp", bufs=2) as encp,
            tc.tile_pool(name="energy", bufs=10) as energy,
            tc.tile_pool(name="smax", bufs=1) as smax,
            tc.tile_pool(name="psum_main", bufs=5, space="PSUM") as psum_main,
            tc.tile_pool(name="psum_v", bufs=2, space="PSUM") as psum_v,
            tc.tile_pool(name="psum_d", bufs=1, space="PSUM") as psum_d,
        ):
            # ---- constants into SBUF ----
            uat_sb = const.tile([128, HC, U], bf16)
            for hc in range(HC):
                nc.sync.dma_start(
                    out=uat_sb[:, hc, :], in_=uat_d[hc * 128:(hc + 1) * 128, :]
                )
            wat_sb = const.tile([128, HAC, U], bf16)
            for hc in range(HAC):
                nc.sync.dma_start(
                    out=wat_sb[:, hc, :], in_=wat_d[hc * 128:(hc + 1) * 128, :]
                )
            dect_sb = const.tile([128, HAC, BL], bf16)
            nc.sync.dma_start(
                out=dect_sb,
                in_=dect_d.ap().rearrange("(c p) b -> p c b", p=128),
            )
            v_sb = const.tile([128, UT], bf16)
            nc.sync.dma_start(out=v_sb, in_=v_d[:, :])

            # ---- D[u, ut*BL + b] = dec-projection + biases, via PE ----
            pd = psum_d.tile([128, UT * BL], f32)
            for ut in range(UT):
                for hc in range(HAC):
                    nc.tensor.matmul(
                        pd[:, ut * BL:(ut + 1) * BL],
                        lhsT=wat_sb[:, hc, ut * 128:(ut + 1) * 128],
                        rhs=dect_sb[:, hc, :],
                        start=(hc == 0),
                        stop=(hc == HAC - 1),
                    )
            d_sb = const.tile([128, UT * BL], f32)
            nc.vector.tensor_copy(out=d_sb, in_=pd)

            att_sb = const.tile([BL, S], f32)

            # ---- main loop over local batches ----
            for bl in range(BL):
                enc_t = encp.tile([128, HC, S], bf16)
                for hc in range(HC):
                    nc.sync.dma_start(
                        out=enc_t[:, hc, :],
                        in_=enc_d[hc * 128:(hc + 1) * 128, bl * S:(bl + 1) * S],
                    )
                ens = []
                for ut in range(UT):
                    ps = psum_main.tile([128, S], f32)
                    for hc in range(HC):
                        nc.tensor.matmul(
                            ps,
                            lhsT=uat_sb[:, hc, ut * 128:(ut + 1) * 128],
                            rhs=enc_t[:, hc, :],
                            start=(hc == 0),
                            stop=(hc == HC - 1),
                        )
                    en = energy.tile([128, S], bf16)
                    col = ut * BL + bl
                    nc.scalar.activation(
                        out=en, in_=ps, func=Tanh,
                        bias=d_sb[:, col:col + 1], scale=1.0,
                    )
                    ens.append(en)
                # v-dot: batch all 8 at the end so PE never waits on ACT
                pv = psum_v.tile([1, S], f32)
                for ut in range(UT):
                    nc.tensor.matmul(
                        pv,
                        lhsT=v_sb[:, ut:ut + 1],
                        rhs=ens[ut],
                        start=(ut == 0),
                        stop=(ut == UT - 1),
                    )
                # engines can only address partition bases 0/32/64/96, and
                # DMA cannot read PSUM: copy to a partition-0 SBUF row, then
                # DMA-gather into partition bl of att_sb
                arow = energy.tile([1, S], mybir.dt.float32, tag="arow")
                nc.vector.tensor_copy(out=arow, in_=pv)
                nc.sync.dma_start(out=att_sb[bl:bl + 1, :], in_=arow)

            # ---- softmax over s (free dim) on [BL, S] ----
            mneg = smax.tile([BL, 1], f32)
            nc.vector.reduce_max(out=mneg, in_=att_sb, axis=X, negate=True)
            ex = smax.tile([BL, S], f32)
            ssum = smax.tile([BL, 1], f32)
            nc.scalar.activation(
                out=ex, in_=att_sb, func=Exp, bias=mneg, scale=1.0,
                accum_out=ssum,
            )
            rsum = smax.tile([BL, 1], f32)
            nc.vector.reciprocal(out=rsum, in_=ssum)
            res = smax.tile([BL, S], f32)
            nc.vector.tensor_scalar_mul(res, ex, rsum)
            nc.sync.dma_start(out=out_d[:, :], in_=res)

    nc.finalize()
    return nc


def _get_nc():
    global _BUILT
    if _BUILT is None:
        _BUILT = _build_bass()
    return _BUILT


def _prep_inputs(encoder_hiddens, last_dec_hidden, Ua_w, Ua_b, Wa_w, Wa_b, v_w):
    """Host-side sharding + layout prep (transpose contraction dims onto
    partitions, cast matmul operands to bf16, fold biases into an extra
    contraction row)."""
    enc = np.asarray(encoder_hiddens, dtype=np.float32)
    dec = np.asarray(last_dec_hidden, dtype=np.float32)
    Ua_w = np.asarray(Ua_w, dtype=np.float32)
    Wa_w = np.asarray(Wa_w, dtype=np.float32)
    bias_u = (np.asarray(Ua_b, np.float32) + np.asarray(Wa_b, np.float32))

    # [S,B,H] -> [H,B,S] bf16 (one big strided copy, then per-core slices
    # are cheap row-contiguous copies)
    enc_t = np.ascontiguousarray(enc.transpose(2, 1, 0)).astype(BF16)

    uat = np.ascontiguousarray(Ua_w.T).astype(BF16)           # [H, U]

    wat_aug = np.zeros((HA, U), np.float32)
    wat_aug[:HD] = Wa_w.T
    wat_aug[HD] = bias_u
    wat_aug = wat_aug.astype(BF16)                            # [HA, U]

    v_prep = np.ascontiguousarray(
        np.asarray(v_w, np.float32).reshape(UT, 128).T
    ).astype(BF16)                                            # [128, UT]

    in_maps = []
    for c in range(NCORES):
        b0 = c * BL
        enc_c = np.ascontiguousarray(
            enc_t[:, b0:b0 + BL, :]
        ).reshape(H, BL * S)                                  # [H, BL*S] bf16

        dect_aug = np.zeros((HA, BL), np.float32)
        dect_aug[:HD] = dec[b0:b0 + BL, :].T
        dect_aug[HD] = 1.0
        dect_aug = dect_aug.astype(BF16)

        in_maps.append({
            "enc": enc_c,
            "uat": uat,
            "wat": wat_aug,
            "dect": dect_aug,
            "v": v_prep,
        })
    return in_maps


def kernel_with_results(**inputs):
    from concourse.bass_utils import run_bass_kernel_spmd

    nc = _get_nc()
    in_maps = _prep_inputs(**inputs)
    res = run_bass_kernel_spmd(nc, in_maps, core_ids=list(range(NCORES)))
    out = np.concatenate(
        [res.results[c]["out"] for c in range(NCORES)], axis=0
    ).astype(np.float32)
    return out, res


def kernel(**inputs):
    out, _ = kernel_with_results(**inputs)
    return out


# revision 5
# speedup vs baseline: 1.0835x; 1.0835x over previous
"""Trainium2 Bass kernel for Bahdanau-style attention.

reference:
    x[s,b,u]  = enc[s,b,:] @ Ua_w[u,:] + Ua_b[u] + dec[b,:] @ Wa_w[u,:] + Wa_b[u]
    att[b,s]  = softmax_s( sum_u v[u] * tanh(x[s,b,u]) )

Sharding: data-parallel over batch. 8 cores x 8 batches each; weights
replicated. All shapes hardcoded (S=512, B=64, H=2048, U=1024).

Per-core device program (bf16 matmul operands, fp32 accumulation):
  1. D[b,u] = dec-projection with Ua_b+Wa_b folded in via an extra
     contraction row (dec.T stationary so weight loads are 8 columns),
     then PE-transposed to D_T[u, ut*8+b] so it can feed the ScalarE
     activation bias port.
  2. for each local batch b (rows are b-major so the dec projection is
     constant along the moving free dim):
       for each u-tile (8 of 128):
         psum[128u, 512s] = sum_hc UaT[hc,utile].T @ enc[hc, b-rows]
         energy = tanh(psum + D_T[:, col])        (ScalarE, bias port)
         acc   += energy * v[utile]               (DVE fused mul-add)
       att[1, 512] = ones.T @ acc                 (PE partition-sum)
       softmax over s on [1, 512], DMA out row b.
"""

import numpy as np
import ml_dtypes

BF16 = ml_dtypes.bfloat16

S = 512          # src len
B = 64           # global batch
H = 2048         # encoder hidden (2*HIDDEN)
HD = 1024        # decoder hidden
U = 1024         # attention units
NCORES = 8
BL = B // NCORES  # local batch per core = 8

HC = H // 128     # 16 h-chunks for main contraction
UT = U // 128     # 8 u-tiles
HA = HD + 128     # augmented dec contraction (1024 + bias row + pad) = 1152
HAC = HA // 128   # 9 chunks

_BUILT = None     # cache so repeated kernel() calls reuse the program


def _build_bass():
    import concourse.mybir as mybir
    from concourse import bacc
    from concourse.tile import TileContext
    from concourse.masks import make_identity

    f32 = mybir.dt.float32
    bf16 = mybir.dt.bfloat16
    Tanh = mybir.ActivationFunctionType.Tanh
    Exp = mybir.ActivationFunctionType.Exp
    X = mybir.AxisListType.X
    MULT = mybir.AluOpType.mult
    ADD = mybir.AluOpType.add

    nc = bacc.Bacc("TRN2", num_devices=NCORES)

    enc_d = nc.dram_tensor("enc", [H, BL * S], bf16, kind="ExternalInput")
    uat_d = nc.dram_tensor("uat", [H, U], bf16, kind="ExternalInput")
    wat_d = nc.dram_tensor("wat", [HA, U], bf16, kind="ExternalInput")
    dect_d = nc.dram_tensor("dect", [HA, BL], bf16, kind="ExternalInput")
    v_d = nc.dram_tensor("v", [128, UT], f32, kind="ExternalInput")
    out_d = nc.dram_tensor("out", [BL, S], f32, kind="ExternalOutput")

    with TileContext(nc) as tc:
        with (
            tc.tile_pool(name="const", bufs=1) as const,
            tc.tile_pool(name="encp", bufs=2) as encp,
            tc.tile_pool(name="energy", bufs=4) as energy,
            tc.tile_pool(name="accp", bufs=3) as accp,
            tc.tile_pool(name="smax", bufs=2) as smax,
            tc.tile_pool(name="psum_main", bufs=4, space="PSUM") as psum_main,
            tc.tile_pool(name="psum_v", bufs=2, space="PSUM") as psum_v,
            tc.tile_pool(name="psum_d", bufs=2, space="PSUM") as psum_d,
        ):
            # ---- constant loads; order = DMA issue order (startup path) ----
            dect_sb = const.tile([128, HAC, BL], bf16)
            nc.sync.dma_start(
                out=dect_sb,
                in_=dect_d.ap().rearrange("(c p) b -> p c b", p=128),
            )
            v_sb = const.tile([128, UT], f32)
            nc.sync.dma_start(out=v_sb, in_=v_d[:, :])
            wat_sb = const.tile([128, HAC, U], bf16)
            for hc in range(HAC):
                nc.sync.dma_start(
                    out=wat_sb[:, hc, :], in_=wat_d[hc * 128:(hc + 1) * 128, :]
                )
            uat_sb = const.tile([128, HC, U], bf16)

            ones_sb = const.tile([128, 1], bf16)
            nc.vector.memset(ones_sb, 1.0)
            id8 = const.tile([8, 8], f32)
            make_identity(nc, id8)

            # ---- D = dec-projection (+folded biases), dec.T stationary ----
            # pdb[b, u-half] accumulated over 9 chunks; weight loads are the
            # tiny 8-column dec.T so PE setup cost is negligible.
            pdb0 = psum_d.tile([8, 512], f32, tag="pd")
            pdb1 = psum_d.tile([8, 512], f32, tag="pd")
            for hc in range(HAC):
                nc.tensor.matmul(
                    pdb0, lhsT=dect_sb[:, hc, :], rhs=wat_sb[:, hc, 0:512],
                    start=(hc == 0), stop=(hc == HAC - 1),
                )
            for hc in range(HAC):
                nc.tensor.matmul(
                    pdb1, lhsT=dect_sb[:, hc, :], rhs=wat_sb[:, hc, 512:1024],
                    start=(hc == 0), stop=(hc == HAC - 1),
                )
            dbu_sb = const.tile([8, U], f32)
            nc.vector.tensor_copy(out=dbu_sb[:, 0:512], in_=pdb0)
            nc.vector.tensor_copy(out=dbu_sb[:, 512:1024], in_=pdb1)
            d_sb = const.tile([128, UT * BL], f32)

            def emit_d_transposes():
                # D[b, u] -> D_T[u-in-tile, ut*8+b] via PE transpose
                pd = psum_d.tile([128, UT * BL], f32, tag="pd")
                for ut in range(UT):
                    nc.tensor.transpose(
                        pd[:, ut * BL:(ut + 1) * BL],
                        in_=dbu_sb[:, ut * 128:(ut + 1) * 128],
                        identity=id8,
                    )
                nc.vector.tensor_copy(out=d_sb, in_=pd)

            def emit_vdot_softmax(bl, acc):
                # att logits: partition-sum of acc via ones-matmul, then
                # per-row softmax entirely on partition 0
                pvs = psum_v.tile([1, S], f32)
                nc.tensor.matmul(pvs, lhsT=ones_sb, rhs=acc,
                                 start=True, stop=True)
                mneg = smax.tile([1, 1], f32)
                nc.vector.reduce_max(out=mneg, in_=pvs, axis=X, negate=True)
                ex = smax.tile([1, S], f32)
                ssum = smax.tile([1, 1], f32)
                nc.scalar.activation(out=ex, in_=pvs, func=Exp, bias=mneg,
                                     scale=1.0, accum_out=ssum)
                rsum = smax.tile([1, 1], f32)
                nc.vector.reciprocal(out=rsum, in_=ssum)
                res = smax.tile([1, S], f32)
                nc.vector.tensor_scalar_mul(res, ex, rsum)
                nc.sync.dma_start(out=out_d[bl:bl + 1, :], in_=res)

            # ---- main loop over local batches ----
            pending = None
            for bl in range(BL):
                enc_t = encp.tile([128, HC, S], bf16)
                for hc in range(HC):
                    if bl == 0:
                        # interleave weight/activation streams so the first
                        # matmul's operands land early
                        nc.sync.dma_start(
                            out=uat_sb[:, hc, :],
                            in_=uat_d[hc * 128:(hc + 1) * 128, :],
                        )
                    nc.sync.dma_start(
                        out=enc_t[:, hc, :],
                        in_=enc_d[hc * 128:(hc + 1) * 128, bl * S:(bl + 1) * S],
                    )
                acc = None
                for ut in range(UT):
                    ps = psum_main.tile([128, S], f32)
                    for hc in range(HC):
                        nc.tensor.matmul(
                            ps,
                            lhsT=uat_sb[:, hc, ut * 128:(ut + 1) * 128],
                            rhs=enc_t[:, hc, :],
                            start=(hc == 0), stop=(hc == HC - 1),
                        )
                    if bl == 0 and ut == 0:
                        # PE reaches these after the first MM block, by which
                        # time the DVE psum->sbuf copies are long done
                        emit_d_transposes()
                    en = energy.tile([128, S], bf16)
                    col = ut * BL + bl
                    nc.scalar.activation(
                        out=en, in_=ps, func=Tanh,
                        bias=d_sb[:, col:col + 1], scale=1.0,
                    )
                    if ut == 0:
                        acc = accp.tile([128, S], bf16)
                        nc.vector.tensor_scalar_mul(acc, en, v_sb[:, 0:1])
                    else:
                        nc.vector.scalar_tensor_tensor(
                            out=acc, in0=en, scalar=v_sb[:, ut:ut + 1],
                            in1=acc, op0=MULT, op1=ADD,
                        )
                if pending is not None:
                    emit_vdot_softmax(*pending)
                pending = (bl, acc)
            emit_vdot_softmax(*pending)

    nc.finalize()
    return nc


def _get_nc():
    global _BUILT
    if _BUILT is None:
        _BUILT = _build_bass()
    return _BUILT


def _prep_inputs(encoder_hiddens, last_dec_hidden, Ua_w, Ua_b, Wa_w, Wa_b, v_w):
    """Host-side sharding + layout prep (transpose contraction dims onto
    partitions, cast matmul operands to bf16, fold biases into an extra
    contraction row)."""
    enc = np.asarray(encoder_hiddens, dtype=np.float32)
    dec = np.asarray(last_dec_hidden, dtype=np.float32)
    Ua_w = np.asarray(Ua_w, dtype=np.float32)
    Wa_w = np.asarray(Wa_w, dtype=np.float32)
    bias_u = (np.asarray(Ua_b, np.float32) + np.asarray(Wa_b, np.float32))

    # [S,B,H] -> [H,B,S] bf16 (one big strided copy, then per-core slices
    # are cheap row-contiguous copies)
    enc_t = np.ascontiguousarray(enc.transpose(2, 1, 0)).astype(BF16)

    uat = np.ascontiguousarray(Ua_w.T).astype(BF16)           # [H, U]

    wat_aug = np.zeros((HA, U), np.float32)
    wat_aug[:HD] = Wa_w.T
    wat_aug[HD] = bias_u
    wat_aug = wat_aug.astype(BF16)                            # [HA, U]

    v_prep = np.ascontiguousarray(
        np.asarray(v_w, np.float32).reshape(UT, 128).T
    )                                                         # [128, UT] f32

    in_maps = []
    for c in range(NCORES):
        b0 = c * BL
        enc_c = np.ascontiguousarray(
            enc_t[:, b0:b0 + BL, :]
        ).reshape(H, BL * S)                                  # [H, BL*S] bf16

        dect_aug = np.zeros((HA, BL), np.float32)
        dect_aug[:HD] = dec[b0:b0 + BL, :].T
        dect_aug[HD] = 1.0
        dect_aug = dect_aug.astype(BF16)

        in_maps.append({
            "enc": enc_c,
            "uat": uat,
            "wat": wat_aug,
            "dect": dect_aug,
            "v": v_prep,
        })
    return in_maps


def kernel_with_results(**inputs):
    from concourse.bass_utils import run_bass_kernel_spmd

    nc = _get_nc()
    in_maps = _prep_inputs(**inputs)
    res = run_bass_kernel_spmd(nc, in_maps, core_ids=list(range(NCORES)))
    out = np.concatenate(
        [res.results[c]["out"] for c in range(NCORES)], axis=0
    ).astype(np.float32)
    return out, res


def kernel(**inputs):
    out, _ = kernel_with_results(**inputs)
    return out
